# revision 1
# baseline (speedup 1.0000x reference)
"""CPE loss kernel for Trainium2, 8 NeuronCores (self-contained).

Strategy: rows sharded 8 ways; each core receives np.roll'ed inputs so its
1024 rows sit first (static diagonal + lhsT slab). Per core: normalized
transposed features (f32r) feed 4 sim matmuls per [128x1024] chunk; the
pos-mask (same label & iou window & flags) is computed as a 25-dim score
matmul; DVE scalar_tensor_tensor ops fuse compare*exp+row-sum; ACT exp
carries the all-pairs sum via accum_out. Host sums the per-core partials.
Measured: rel err ~2.4e-6 vs reference; ~240 us HW per execution
(~100 us preprocessing + ~142 us main loop, cost-model + HW reps-delta).
"""

import sys

sys.path.insert(0, "/opt/trn_rl_repo")

from contextlib import ExitStack

import numpy as np

import concourse.bacc as bacc
import concourse.tile as tile
from concourse import mybir

AF = mybir.ActivationFunctionType
ALU = mybir.AluOpType
AX = mybir.AxisListType
F32 = mybir.dt.float32
F32R = mybir.dt.float32r
BF16 = mybir.dt.bfloat16
I32 = mybir.dt.int32

N = 8192
D = 256
NCORES = 8
RPC = N // NCORES  # 1024 rows per core
P = 128
MT = RPC // P  # 8 row-tiles per core
KT = D // P  # 2 contraction tiles
AT = N // P  # 64 column stat tiles

TAU = 0.1
POS_T = 0.5
NEG_T = 0.3
IOU_DIFF_T = 0.2
SQRT_INV_TAU = float(np.sqrt(1.0 / TAU))

NLAB = 21  # labels -1..19
SD = NLAB + 4  # score contraction dims = 25
BIG = 1000.0
KILL = 1.0e6
COL_KILL_L = -512.0
ROW_KILL_L = -640.0
DIAG_KILL = -300.0
SCORE_T = IOU_DIFF_T * IOU_DIFF_T  # 0.04

CHUNK = 1024
NCH = N // CHUNK  # 8
DT_E = BF16
SAME_ENGINE = "vector"  # "vector" | "gpsimd"
GROUP = 16  # preproc load group size (default)


def build(reps=1, same_engine=None, dt_e=None, chunk=None, no_score=False, skip_onehot=False, group=None, ssq_bufs=2, tsame_frac=0.0):
    same_engine = same_engine or SAME_ENGINE
    dt_e = dt_e or DT_E
    chunk = chunk or CHUNK
    grp = group or GROUP
    nch = N // chunk
    nh = chunk // 512  # 512-wide matmuls per chunk

    nc = bacc.Bacc("TRN2", target_bir_lowering=False)

    feats = nc.dram_tensor("feats", [N, D], F32, kind="ExternalInput")
    labels = nc.dram_tensor("labels", [N], I32, kind="ExternalInput")
    ious = nc.dram_tensor("ious", [N], F32, kind="ExternalInput")

    out_loss = nc.dram_tensor("out_loss", [P, MT], F32, kind="ExternalOutput")
    out_valid = nc.dram_tensor("out_valid", [P, MT], F32, kind="ExternalOutput")
    out_dbg = nc.dram_tensor("out_dbg", [P, 3 * MT], F32, kind="ExternalOutput")

    scr_l = nc.dram_tensor("scr_l", [N], BF16, kind="Internal")
    scr_k = nc.dram_tensor("scr_k", [1], F32, kind="Internal")
    # score vectors in [SD, N] layout, built via DRAM round-trip
    scr_sc = nc.dram_tensor("scr_sc", [SD, N], F32, kind="Internal")
    scr_sr = nc.dram_tensor("scr_sr", [SD, N], F32, kind="Internal")

    with tile.TileContext(nc) as tc, ExitStack() as ctx:
        cpool = ctx.enter_context(tc.tile_pool(name="const", bufs=1))
        spool = ctx.enter_context(tc.tile_pool(name="stats", bufs=1))
        xtpool = ctx.enter_context(tc.tile_pool(name="xt", bufs=1))
        wpool = ctx.enter_context(tc.tile_pool(name="work", bufs=3))
        accpool = ctx.enter_context(tc.tile_pool(name="acc", bufs=1))
        fpool = ctx.enter_context(tc.tile_pool(name="final", bufs=1))
        # ---- constants ----------------------------------------------------
        eye = cpool.tile([P, P], F32)
        nc.vector.memset(eye, 1.0)
        nc.gpsimd.affine_select(
            out=eye[:, :], in_=eye[:, :], pattern=[[-1, P]],
            compare_op=ALU.is_equal, fill=0.0, base=0, channel_multiplier=1,
        )
        ones_col = cpool.tile([P, 1], F32)
        nc.vector.memset(ones_col, 1.0)

        # ---- column stats [P, AT] -----------------------------------------
        lab_i32 = spool.tile([P, AT], I32)
        nc.sync.dma_start(out=lab_i32, in_=labels.rearrange("(a p) -> p a", p=P))
        iou_c = spool.tile([P, AT], F32)
        nc.sync.dma_start(out=iou_c, in_=ious.rearrange("(a p) -> p a", p=P))
        lab_c = spool.tile([P, AT], F32)
        nc.vector.tensor_copy(lab_c[:, :], lab_i32[:, :])

        fg_c = spool.tile([P, AT], F32)
        nc.vector.tensor_scalar(
            out=fg_c[:, :], in0=lab_c[:, :], scalar1=0.0, scalar2=None, op0=ALU.is_ge
        )
        posflag_c = spool.tile([P, AT], F32)
        nc.vector.scalar_tensor_tensor(
            out=posflag_c[:, :], in0=iou_c[:, :], scalar=POS_T, in1=fg_c[:, :],
            op0=ALU.is_gt, op1=ALU.mult,
        )
        # centered iou and its square
        iou_cc = spool.tile([P, AT], F32)
        nc.vector.tensor_scalar(
            out=iou_cc[:, :], in0=iou_c[:, :], scalar1=-0.5, scalar2=None, op0=ALU.add
        )
        iou_sq = spool.tile([P, AT], F32)
        nc.vector.tensor_tensor(
            out=iou_sq[:, :], in0=iou_cc[:, :], in1=iou_cc[:, :], op=ALU.mult
        )
        # kill = KILL * (1 - posflag)
        kill_c = spool.tile([P, AT], F32)
        nc.vector.tensor_scalar(
            out=kill_c[:, :], in0=posflag_c[:, :], scalar1=-KILL, scalar2=KILL,
            op0=ALU.mult, op1=ALU.add,
        )

        # L_fg (bf16): fg ? label : COL_KILL_L,  exact select
        t1_l = spool.tile([P, AT], F32)
        nc.vector.tensor_tensor(out=t1_l[:, :], in0=lab_c[:, :], in1=fg_c[:, :], op=ALU.mult)
        t2_l = spool.tile([P, AT], F32)
        nc.vector.tensor_scalar(
            out=t2_l[:, :], in0=fg_c[:, :], scalar1=-COL_KILL_L, scalar2=COL_KILL_L,
            op0=ALU.mult, op1=ALU.add,
        )
        lfg_c = spool.tile([P, AT], BF16)
        nc.vector.tensor_tensor(out=lfg_c[:, :], in0=t1_l[:, :], in1=t2_l[:, :], op=ALU.add)
        # row variant: fg ? label : ROW_KILL_L
        t2r_l = spool.tile([P, AT], F32)
        nc.vector.tensor_scalar(
            out=t2r_l[:, :], in0=fg_c[:, :], scalar1=-ROW_KILL_L, scalar2=ROW_KILL_L,
            op0=ALU.mult, op1=ALU.add,
        )
        labfg_r = spool.tile([P, MT], F32)
        nc.vector.tensor_tensor(
            out=labfg_r[:, :], in0=t1_l[:, :MT], in1=t2r_l[:, :MT], op=ALU.add
        )
        nr_r = spool.tile([P, MT], F32)
        nc.vector.tensor_scalar(
            out=nr_r[:, :], in0=iou_c[:, :MT], scalar1=NEG_T, scalar2=None,
            op0=ALU.is_ge,
        )
        fg_r = spool.tile([P, MT], F32)
        nc.vector.tensor_copy(fg_r[:, :], fg_c[:, :MT])

        # ---- score vectors -------------------------------------------------
        # Unified column basis u_col [27, N]:
        #   0..20 : -B * onehot_j           21: 1          22: 1
        #   23    : iou'^2_j + poskill_j    24: iou'_j
        #   25    : -KILL*(1-fg_j)          26: 1
        # pos-score  = sc_row[25] . u_col[0:25]
        #   sc_row: [oh_i, B, iou'^2_i + poskill_i, 1, -2*iou'_i]
        # samescore  = ss_row[27] . u_col[0:27]  (dims 22..24 zero)
        #   ss_row: [-oh_i, -B, 0, 0, 0, 1, -KILL*(1-fg_i)]
        SDU = SD + 2  # 27
        scb = ctx.enter_context(tc.tile_pool(name="scbuild", bufs=1))
        u_col = scb.tile([P, AT, SDU], F32)
        sc_row = scb.tile([P, AT, SD], F32)
        ss_row = scb.tile([P, AT, SDU], F32)
        nc.vector.memset(ss_row[:, :, :], 0.0)
        for l in range(NLAB):
            oh = scb.tile([P, AT], F32, tag="oh_tmp")
            nc.vector.tensor_scalar(
                out=oh[:, :], in0=lab_c[:, :], scalar1=float(l - 1), scalar2=None,
                op0=ALU.is_equal,
            )
            nc.vector.tensor_scalar(
                out=u_col[:, :, l], in0=oh[:, :], scalar1=-BIG, scalar2=None,
                op0=ALU.mult,
            )
            nc.vector.tensor_copy(sc_row[:, :, l], oh[:, :])
            nc.vector.tensor_scalar(
                out=ss_row[:, :, l], in0=oh[:, :], scalar1=-1.0, scalar2=None,
                op0=ALU.mult,
            )
        # kill = KILL * (1 - posflag) reused; fg kill for samescore
        fgkill_c = spool.tile([P, AT], F32)
        nc.vector.tensor_scalar(
            out=fgkill_c[:, :], in0=fg_c[:, :], scalar1=KILL, scalar2=-KILL,
            op0=ALU.mult, op1=ALU.add,
        )
        nc.vector.memset(u_col[:, :, NLAB], 1.0)
        nc.vector.memset(u_col[:, :, NLAB + 1], 1.0)
        nc.vector.tensor_tensor(
            out=u_col[:, :, NLAB + 2], in0=iou_sq[:, :], in1=kill_c[:, :], op=ALU.add
        )
        nc.vector.tensor_copy(u_col[:, :, NLAB + 3], iou_cc[:, :])
        nc.vector.tensor_copy(u_col[:, :, NLAB + 4], fgkill_c[:, :])
        nc.vector.memset(u_col[:, :, NLAB + 5], 1.0)

        nc.vector.memset(sc_row[:, :, NLAB], BIG)
        nc.vector.tensor_tensor(
            out=sc_row[:, :, NLAB + 1], in0=iou_sq[:, :], in1=kill_c[:, :], op=ALU.add
        )
        nc.vector.memset(sc_row[:, :, NLAB + 2], 1.0)
        nc.vector.tensor_scalar(
            out=sc_row[:, :, NLAB + 3], in0=iou_cc[:, :], scalar1=-2.0, scalar2=None,
            op0=ALU.mult,
        )

        nc.vector.memset(ss_row[:, :, NLAB], -BIG)
        nc.vector.memset(ss_row[:, :, NLAB + 4], 1.0)
        nc.vector.tensor_copy(ss_row[:, :, NLAB + 5], fgkill_c[:, :])

        u_col_r = xtpool.tile([SDU, N], F32R)
        sc_row_r = xtpool.tile([SD, RPC], F32R)
        ss_row_r = xtpool.tile([SDU, RPC], F32R)
        with tc.tile_pool(name="psum_scb", bufs=3, space="PSUM") as pscb:
            for a in range(AT):
                pst1 = pscb.tile([SDU, P], F32, tag="sctp")
                nc.tensor.transpose(pst1[:, :], u_col[:, a, :], eye[:, :])
                if a % 2 == 0:
                    nc.scalar.copy(u_col_r[:, a * P : (a + 1) * P], pst1[:, :])
                else:
                    nc.vector.tensor_copy(u_col_r[:, a * P : (a + 1) * P], pst1[:, :])
                if a < MT:
                    pst2 = pscb.tile([SD, P], F32, tag="sctp")
                    nc.tensor.transpose(pst2[:, :], sc_row[:, a, :], eye[:, :])
                    nc.vector.tensor_copy(sc_row_r[:, a * P : (a + 1) * P], pst2[:, :])
                    pst4 = pscb.tile([SDU, P], F32, tag="sctp")
                    nc.tensor.transpose(pst4[:, :], ss_row[:, a, :], eye[:, :])
                    nc.scalar.copy(ss_row_r[:, a * P : (a + 1) * P], pst4[:, :])

        # n_nonfg = N - sum(fg), broadcast to [P, 1]
        fg_red = spool.tile([P, 1], F32)
        nc.vector.tensor_reduce(out=fg_red[:, :], in_=fg_c[:, :], axis=AX.X, op=ALU.add)
        with tc.tile_pool(name="psum_k", bufs=1, space="PSUM") as pskpool:
            ps_k = pskpool.tile([1, 1], F32)
            nc.tensor.matmul(
                ps_k[:, :], fg_red[:, :], ones_col[:, :], start=True, stop=True
            )
            k_sb = spool.tile([1, 1], F32)
            nc.vector.tensor_scalar(
                out=k_sb[:, :], in0=ps_k[:, :], scalar1=-1.0, scalar2=float(N),
                op0=ALU.mult, op1=ALU.add,
            )
        nc.sync.dma_start(out=scr_k[:, None], in_=k_sb[:, :])
        knf_b = spool.tile([P, 1], F32)
        nc.sync.dma_start(out=knf_b[:, :], in_=scr_k[None, :].to_broadcast([P, 1]))

        # broadcast L_fg row to [P, N]
        nc.sync.dma_start(out=scr_l.rearrange("(a p) -> p a", p=P), in_=lfg_c[:, :])
        L_fg = xtpool.tile([P, N], BF16)
        nc.sync.dma_start(out=L_fg[:, :], in_=scr_l[None, :].to_broadcast([P, N]))

        # ---- single-pass normalize + transpose -> XT f32r ------------------
        xt_full = [xtpool.tile([P, N], F32R, name=f"xt{k}", tag=f"xt{k}") for k in range(KT)]
        # feats viewed so one DMA loads a whole group: [P, grp, D]
        feats_g = feats.rearrange("(a p) d -> p a d", p=P)
        with tc.tile_pool(name="load", bufs=2) as ldpool, tc.tile_pool(
            name="norm", bufs=3
        ) as npool, tc.tile_pool(
            name="psum_t", bufs=4, space="PSUM"
        ) as pstpool:
            for g in range(AT // grp):
                xg = ldpool.tile([P, grp, D], F32, tag="xload")
                nc.sync.dma_start(
                    out=xg, in_=feats_g[:, g * grp : (g + 1) * grp, :]
                )
                ssq_g = spool.tile([P, grp], F32, tag="ssq_g", bufs=ssq_bufs)
                for ai in range(grp):
                    sq_scr = npool.tile([P, D], F32, tag="sq_scr")
                    if ai % 2 == 0:
                        nc.vector.scalar_tensor_tensor(
                            out=sq_scr[:, :], in0=xg[:, ai, :], scalar=1.0,
                            in1=xg[:, ai, :], op0=ALU.mult, op1=ALU.mult,
                            accum_out=ssq_g[:, ai : ai + 1],
                        )
                    else:
                        nc.scalar.activation(
                            sq_scr[:, :], xg[:, ai, :], AF.Square, bias=0.0,
                            scale=1.0, accum_out=ssq_g[:, ai : ai + 1],
                        )
                s_g = npool.tile([P, grp], F32, tag="s_g")
                nc.scalar.activation(s_g[:, :], ssq_g[:, :], AF.Sqrt, bias=0.0, scale=1.0)
                y_g = npool.tile([P, grp], F32, tag="y_g")
                nc.vector.reciprocal(y_g[:, :], s_g[:, :])
                y2_g = npool.tile([P, grp], F32, tag="y2_g")
                nc.vector.tensor_tensor(out=y2_g[:, :], in0=y_g[:, :], in1=y_g[:, :], op=ALU.mult)
                h_g = npool.tile([P, grp], F32, tag="h_g")
                nc.vector.tensor_tensor(out=h_g[:, :], in0=ssq_g[:, :], in1=y2_g[:, :], op=ALU.mult)
                nc.vector.tensor_scalar(
                    out=h_g[:, :], in0=h_g[:, :], scalar1=-0.5, scalar2=1.5,
                    op0=ALU.mult, op1=ALU.add,
                )
                rn_g = npool.tile([P, grp], F32, tag="rn_g")
                nc.vector.tensor_tensor(out=rn_g[:, :], in0=y_g[:, :], in1=h_g[:, :], op=ALU.mult)
                rnf_g = npool.tile([P, grp], F32, tag="rnf_g")
                nc.vector.scalar_tensor_tensor(
                    out=rnf_g[:, :], in0=rn_g[:, :], scalar=SQRT_INV_TAU,
                    in1=fg_c[:, g * grp : (g + 1) * grp],
                    op0=ALU.mult, op1=ALU.mult,
                )
                for ai in range(grp):
                    a = g * grp + ai
                    xn = npool.tile([P, D], F32, tag="xnorm")
                    nc.vector.tensor_scalar(
                        out=xn[:, :], in0=xg[:, ai, :], scalar1=rnf_g[:, ai : ai + 1],
                        scalar2=None, op0=ALU.mult,
                    )
                    for k in range(KT):
                        pst = pstpool.tile([P, P], F32, tag="pst")
                        nc.tensor.transpose(
                            pst[:, :], xn[:, k * P : (k + 1) * P], eye[:, :]
                        )
                        if (a * KT + k) % 2 == 0:
                            nc.scalar.copy(xt_full[k][:, a * P : (a + 1) * P], pst[:, :])
                        else:
                            nc.vector.tensor_copy(
                                xt_full[k][:, a * P : (a + 1) * P], pst[:, :]
                            )

        # ---- main loop ----------------------------------------------------
        pspool = ctx.enter_context(tc.tile_pool(name="psum", bufs=2, space="PSUM"))
        pscpool = ctx.enter_context(tc.tile_pool(name="psum_sc", bufs=2, space="PSUM"))
        pos_slots = [accpool.tile([P, nch], F32, name=f"pos_s{m}", tag=f"pos_s{m}") for m in range(MT)]
        t_slots = [accpool.tile([P, nch], F32, name=f"t_s{m}", tag=f"t_s{m}") for m in range(MT)]
        s_slots = [accpool.tile([P, nch], F32, name=f"s_s{m}", tag=f"s_s{m}") for m in range(MT)]

        same_eng = nc.vector if same_engine == "vector" else nc.gpsimd

        for _rep in range(reps):
            for m in range(MT):
                for jc in range(nch):
                    hybrid = (jc + m) % 100 < int(tsame_frac * 100)
                    ps = pspool.tile([P, chunk], F32, tag="sim")
                    for k in range(KT):
                        for h in range(nh):
                            nc.tensor.matmul(
                                ps[:, h * 512 : (h + 1) * 512],
                                xt_full[k][:, m * P : (m + 1) * P],
                                xt_full[k][
                                    :, jc * chunk + h * 512 : jc * chunk + (h + 1) * 512
                                ],
                                start=(k == 0),
                                stop=(k == KT - 1),
                            )
                    if not no_score:
                        psc = pscpool.tile([P, chunk], F32, tag="score")
                        for h in range(nh):
                            nc.tensor.matmul(
                                psc[:, h * 512 : (h + 1) * 512],
                                sc_row_r[:, m * P : (m + 1) * P],
                                u_col_r[:SD, jc * chunk + h * 512 : jc * chunk + (h + 1) * 512],
                                start=True,
                                stop=True,
                            )
                    if jc == (m * P) // chunk:
                        doff = (m * P) % chunk
                        nc.vector.scalar_tensor_tensor(
                            out=ps[:, doff : doff + P],
                            in0=eye[:, :], scalar=DIAG_KILL,
                            in1=ps[:, doff : doff + P],
                            op0=ALU.mult, op1=ALU.add,
                        )
                    e_t = wpool.tile([P, chunk], dt_e, tag="e")
                    nc.scalar.activation(
                        e_t[:, :], ps[:, :], AF.Exp, bias=0.0, scale=1.0,
                        accum_out=s_slots[m][:, jc : jc + 1],
                    )
                    sc1 = wpool.tile([P, chunk], dt_e, tag="sc1")
                    nc.vector.scalar_tensor_tensor(
                        out=sc1[:, :],
                        in0=(psc[:, :] if not no_score else L_fg[:, jc * chunk : (jc + 1) * chunk]),
                        scalar=SCORE_T, in1=e_t[:, :],
                        op0=ALU.is_lt, op1=ALU.mult,
                        accum_out=pos_slots[m][:, jc : jc + 1],
                    )
                    if hybrid:
                        # accumulate samescore into the sim psum after e is
                        # read, then T_same = sum(exp(psum2)) on ACT
                        for h in range(nh):
                            nc.tensor.matmul(
                                ps[:, h * 512 : (h + 1) * 512],
                                ss_row_r[:, m * P : (m + 1) * P],
                                u_col_r[:, jc * chunk + h * 512 : jc * chunk + (h + 1) * 512],
                                start=False,
                                stop=True,
                                skip_group_check=True,
                            )
                        e2_t = wpool.tile([P, chunk], dt_e, tag="sc2")
                        nc.scalar.activation(
                            e2_t[:, :], ps[:, :], AF.Exp, bias=0.0, scale=1.0,
                            accum_out=t_slots[m][:, jc : jc + 1],
                        )
                    else:
                        sc2 = wpool.tile([P, chunk], dt_e, tag="sc2")
                        same_eng.scalar_tensor_tensor(
                            out=sc2[:, :], in0=L_fg[:, jc * chunk : (jc + 1) * chunk],
                            scalar=labfg_r[:, m : m + 1], in1=e_t[:, :],
                            op0=ALU.is_equal, op1=ALU.mult,
                            accum_out=t_slots[m][:, jc : jc + 1],
                        )

        # ---- finalize ------------------------------------------------------
        pos_c = fpool.tile([P, MT], F32)
        t_c = fpool.tile([P, MT], F32)
        s_c = fpool.tile([P, MT], F32)
        for m in range(MT):
            nc.vector.tensor_reduce(
                out=pos_c[:, m : m + 1], in_=pos_slots[m][:, :], axis=AX.X, op=ALU.add
            )
            nc.vector.tensor_reduce(
                out=t_c[:, m : m + 1], in_=t_slots[m][:, :], axis=AX.X, op=ALU.add
            )
            nc.vector.tensor_reduce(
                out=s_c[:, m : m + 1], in_=s_slots[m][:, :], axis=AX.X, op=ALU.add
            )

        sfg = fpool.tile([P, MT], F32)
        nc.vector.tensor_scalar(
            out=sfg[:, :], in0=s_c[:, :], scalar1=knf_b[:, :], scalar2=None,
            op0=ALU.subtract,
        )
        nc.vector.tensor_tensor(out=sfg[:, :], in0=sfg[:, :], in1=fg_r[:, :], op=ALU.mult)
        tn = fpool.tile([P, MT], F32)
        nc.vector.tensor_tensor(out=tn[:, :], in0=t_c[:, :], in1=nr_r[:, :], op=ALU.mult)
        neg_c = fpool.tile([P, MT], F32)
        nc.vector.tensor_tensor(out=neg_c[:, :], in0=sfg[:, :], in1=tn[:, :], op=ALU.subtract)
        all_c = fpool.tile([P, MT], F32)
        nc.vector.tensor_tensor(out=all_c[:, :], in0=pos_c[:, :], in1=neg_c[:, :], op=ALU.add)

        pos_safe = fpool.tile([P, MT], F32)
        nc.vector.tensor_scalar(
            out=pos_safe[:, :], in0=pos_c[:, :], scalar1=1e-30, scalar2=None, op0=ALU.max
        )
        rp = fpool.tile([P, MT], F32)
        nc.vector.reciprocal(rp[:, :], pos_safe[:, :])
        ratio = fpool.tile([P, MT], F32)
        nc.vector.tensor_tensor(out=ratio[:, :], in0=all_c[:, :], in1=rp[:, :], op=ALU.mult)
        nc.vector.tensor_scalar(
            out=ratio[:, :], in0=ratio[:, :], scalar1=1.0, scalar2=None, op0=ALU.max
        )
        lg = fpool.tile([P, MT], F32)
        nc.scalar.activation(lg[:, :], ratio[:, :], AF.Ln, bias=0.0, scale=1.0)
        nc.vector.tensor_scalar(
            out=lg[:, :], in0=lg[:, :], scalar1=10.0, scalar2=None, op0=ALU.min
        )

        vp = fpool.tile([P, MT], F32)
        nc.vector.tensor_scalar(
            out=vp[:, :], in0=pos_c[:, :], scalar1=0.0, scalar2=None, op0=ALU.is_gt
        )
        vn = fpool.tile([P, MT], F32)
        nc.vector.tensor_scalar(
            out=vn[:, :], in0=neg_c[:, :], scalar1=0.0, scalar2=None, op0=ALU.is_gt
        )
        valid = fpool.tile([P, MT], F32)
        nc.vector.tensor_tensor(out=valid[:, :], in0=vp[:, :], in1=vn[:, :], op=ALU.mult)
        nc.vector.tensor_tensor(out=valid[:, :], in0=valid[:, :], in1=fg_r[:, :], op=ALU.mult)

        lossm = fpool.tile([P, MT], F32)
        nc.vector.tensor_tensor(out=lossm[:, :], in0=lg[:, :], in1=valid[:, :], op=ALU.mult)

        nc.sync.dma_start(out=out_loss[:, :], in_=lossm[:, :])
        nc.sync.dma_start(out=out_valid[:, :], in_=valid[:, :])
        nc.sync.dma_start(out=out_dbg[:, 0:MT], in_=pos_c[:, :])
        nc.sync.dma_start(out=out_dbg[:, MT : 2 * MT], in_=t_c[:, :])
        nc.sync.dma_start(out=out_dbg[:, 2 * MT : 3 * MT], in_=s_c[:, :])

    nc.compile()
    return nc


_NC_CACHE = {}


def get_nc(reps=1, **kw):
    key = (reps, tuple(sorted(kw.items())))
    if key not in _NC_CACHE:
        _NC_CACHE[key] = build(reps, **kw)
    return _NC_CACHE[key]


def make_in_maps(feats, ious, labels):
    feats = np.ascontiguousarray(feats, dtype=np.float32)
    ious = np.ascontiguousarray(ious, dtype=np.float32)
    labels = np.ascontiguousarray(labels, dtype=np.int32)
    in_maps = []
    for c in range(NCORES):
        sh = c * RPC
        in_maps.append(
            {
                "feats": np.roll(feats, -sh, axis=0),
                "labels": np.roll(labels, -sh),
                "ious": np.roll(ious, -sh),
            }
        )
    return in_maps


def finalize(results):
    total = 0.0
    cnt = 0.0
    for r in results:
        total += float(r["out_loss"].astype(np.float64).sum())
        cnt += float(r["out_valid"].astype(np.float64).sum())
    if cnt > 0:
        return np.float32(np.float32(total) / np.float32(max(cnt, 1.0)))
    return np.float32(0.0)


def kernel(proposal_features, proposal_ious, proposal_labels, reps=1, **kw):
    """Full inputs in, full (scalar) output out. Shards internally across
    the 8 NeuronCores and reduces the per-core partial sums on the host."""
    from concourse.bass_utils import run_bass_kernel_spmd

    nc = get_nc(reps, **kw)
    in_maps = make_in_maps(proposal_features, proposal_ious, proposal_labels)
    res = run_bass_kernel_spmd(nc, in_maps, core_ids=list(range(NCORES)))
    return finalize(res.results)

